# revision 17
# baseline (speedup 1.0000x reference)
"""ArgMaxTop Trainium2 kernel.

Math: out[b] = argmax_c sum_s x[b,s,c] * [x[b,s,c] >= t(b,s)] where t is the
8th-largest value of row (b,s). This equals the reference's
scatter-top8/mean/argmax pipeline for inputs without exact float ties (the
mean-over-s divides every class by the same S, so argmax is unchanged; absent
classes sum to 0 < any winner).

Sharding: batch b -> core b (8 batches, 8 cores), no collectives.

Per-core dataflow, per 128-row s-tile (16 tiles):
  - DMA x chunks [128, 8000] f32 to SBUF (pool of 5 bufs for overlap)
  - DVE `max` (top-8 per partition) per chunk -> concat [128,32] -> max -> t
  - value stream (all classes, ACT): r = Relu(x - t), f32r
  - selection stream, split across engines to balance them (DVE also owns
    the max8 pass; GpSimd is a DSP, useless for elementwise):
      * DVE windows: m2 = (x >= t) * t, f32r (fused tensor_scalar, 2x mode)
      * ACT windows: s = Sign(x - t') in f32r ({-1,+1}, exact); t' =
        fl(t*(1-2^-23)) sits strictly between the 9th and 8th largest for
        every row of this input (verified: no x in [t', t)), so s = 2m-1
        exactly; contracted against an f32r t/2 stationary this contributes
        t*m - t/2 per row. The -t/2 shift is class-independent; it is
        measured per tile (tsum output) and added back on the host.
  - PE: per 500-wide class window j, two matmuls accumulate into PSUM
    [64,500] via shifted-window one-hot stationaries (ones in column 64):
      psum += ones_j^T @ r  +  Wsel_j^T @ sel
    where Wsel = ones for m2 windows and the per-tile t/2 column (W2,
    double-buffered by tile parity) for sign windows.
  - drain psum -> SBUF -> DRAM out [64, 500] + tsum [1,16] (per-tile
    sum_s t_s/2, fp32); host adds T_half to the sign-path classes and
    argmaxes the 32000 sums.
"""

import sys

if "/opt/trn_rl_repo" not in sys.path:
    sys.path.insert(0, "/opt/trn_rl_repo")

import numpy as np

B, S, C = 8, 2048, 32000
TOP_K = 8
P = 128          # partitions per s-tile
XCH = 8000       # x chunk width (DVE max input free size <= 16384)
WCH = 2000       # relu/sign/mask window width
RCH = 1000       # relu (ACT) window width
CCH = 500        # matmul moving window / psum columns
NROWS = C // CCH  # 64 psum rows
NTILES = S // P
NXCH = C // XCH
NW = XCH // WCH  # 4 select windows per x chunk
# per chunk j: sign path covers the first SIGN_WINDOWS[j] of its 4 windows
SIGN_WINDOWS = [2, 1, 2, 1]

_CACHE = {}


def _build_graph(s_len=S, x_bufs=5):
    from concourse import bacc, tile, mybir

    f32 = mybir.dt.float32
    f32r = mybir.dt.float32r
    Alu = mybir.AluOpType
    Act = mybir.ActivationFunctionType

    nc = bacc.Bacc("TRN2", target_bir_lowering=False, debug=False)
    x = nc.dram_tensor("x", [s_len, C], f32, kind="ExternalInput").ap()
    zc = nc.dram_tensor("zcols", [P, P], f32, kind="ExternalInput").ap()
    out = nc.dram_tensor("out", [NROWS, CCH], f32, kind="ExternalOutput").ap()
    ntiles = s_len // P
    tsum = nc.dram_tensor("tsum", [1, ntiles], f32, kind="ExternalOutput").ap()

    n_mm = ntiles * (C // CCH) * 2
    mm_i = 0
    SCALE_P = float(np.float32(1.0) - np.float32(2.0**-23))

    with tile.TileContext(nc) as tc:
        with (
            tc.tile_pool(name="consts", bufs=1) as consts,
            tc.tile_pool(name="xp", bufs=x_bufs) as xp,
            tc.tile_pool(name="tp", bufs=3) as tp,
            tc.tile_pool(name="rp", bufs=3) as rp,
            tc.tile_pool(name="selp", bufs=3) as selp,
            tc.tile_pool(name="sump", bufs=1) as sump,
            tc.tile_pool(name="ps", bufs=1, space="PSUM") as ps,
            tc.tile_pool(name="ps2", bufs=1, space="PSUM") as ps2,
        ):
            zt = consts.tile([P, P], f32, name="zt")
            nc.sync.dma_start(out=zt, in_=zc)
            ztm = consts.tile([P, P], f32r, name="ztm")
            nc.vector.tensor_copy(ztm, zt)
            w2 = [
                consts.tile([P, P], f32r, name=f"w2_{k}") for k in range(2)
            ]
            for w in w2:
                # zeros except col 64, which every tile overwrites with t/2
                # before use (f32r memset fails the ISA check, so copy-convert)
                nc.vector.tensor_copy(w, zt)

            acc = ps.tile([NROWS, CCH], f32, name="acc")
            acc2 = ps2.tile([1, ntiles], f32, name="acc2")

            for it in range(ntiles):
                xch = []
                for j in range(NXCH):
                    xt = xp.tile([P, XCH], f32, name="xch", tag="xch")
                    nc.sync.dma_start(
                        out=xt,
                        in_=x[it * P : (it + 1) * P, j * XCH : (j + 1) * XCH],
                    )
                    xch.append(xt)

                top = tp.tile([P, 8 * NXCH], f32, name="top", tag="top")
                for j in range(NXCH):
                    nc.vector.max(out=top[:, 8 * j : 8 * (j + 1)], in_=xch[j])
                top8 = tp.tile([P, 8], f32, name="top8", tag="top8")
                nc.vector.max(out=top8, in_=top)
                t_ap = top8[:, 7:8]

                # per-tile scalars: relu bias, sign bias, t/2 column
                tneg = tp.tile([P, 1], f32, name="tneg", tag="tneg")
                nc.scalar.activation(
                    out=tneg, in_=t_ap, func=Act.Copy, scale=-1.0
                )
                tpneg = tp.tile([P, 1], f32, name="tpneg", tag="tpneg")
                nc.scalar.activation(
                    out=tpneg, in_=t_ap, func=Act.Copy, scale=-SCALE_P
                )
                t2col = w2[it % 2][:, 64:65]
                nc.scalar.activation(
                    out=t2col, in_=t_ap, func=Act.Copy, scale=0.5
                )

                # per-tile T_half = sum_s t_s/2 (own psum column, host-summed;
                # plain fp32 1-col matmul — f32r rejects 1-wide moving)
                t2f = tp.tile([P, 1], f32, name="t2f", tag="t2f")
                nc.scalar.activation(out=t2f, in_=t2col, func=Act.Copy)
                nc.tensor.matmul(
                    acc2[:, it : it + 1],
                    zt[:, 64:65],
                    t2f,
                    start=True,
                    stop=True,
                )

                for j in range(NXCH):
                    nsign = SIGN_WINDOWS[j]
                    for wi in range(NW):
                        xs = xch[j][:, wi * WCH : (wi + 1) * WCH]
                        if wi < nsign:
                            sel = selp.tile(
                                [P, WCH], f32r, name="sel", tag="sel"
                            )
                            nc.scalar.activation(
                                out=sel,
                                in_=xs,
                                func=Act.Sign,
                                bias=tpneg,
                                scale=1.0,
                            )
                            wsel = w2[it % 2]
                        else:
                            sel = selp.tile(
                                [P, WCH], f32r, name="sel", tag="sel"
                            )
                            nc.vector.tensor_scalar(
                                sel, xs, t_ap, t_ap, Alu.is_ge, Alu.mult
                            )
                            wsel = ztm
                        rtiles = []
                        for rs in range(WCH // RCH):
                            off = wi * WCH + rs * RCH
                            r = rp.tile([P, RCH], f32r, name="r", tag="r")
                            nc.scalar.activation(
                                out=r,
                                in_=xch[j][:, off : off + RCH],
                                func=Act.Relu,
                                bias=tneg,
                                scale=1.0,
                            )
                            rtiles.append(r)
                        for w in range(WCH // CCH):
                            cg = (j * XCH + wi * WCH + w * CCH) // CCH
                            off = w * CCH
                            rsrc = rtiles[off // RCH][
                                :, off % RCH : off % RCH + CCH
                            ]
                            nc.tensor.matmul(
                                acc,
                                ztm[:, 64 - cg : 128 - cg],
                                rsrc,
                                start=(mm_i == 0),
                                stop=(mm_i == n_mm - 1),
                            )
                            mm_i += 1
                            nc.tensor.matmul(
                                acc,
                                wsel[:, 64 - cg : 128 - cg],
                                sel[:, off : off + CCH],
                                start=False,
                                stop=(mm_i == n_mm - 1),
                            )
                            mm_i += 1

            sums = sump.tile([NROWS, CCH], f32, name="sums")
            nc.scalar.activation(out=sums, in_=acc, func=Act.Copy)
            nc.sync.dma_start(out=out, in_=sums)
            tsums = sump.tile([1, ntiles], f32, name="tsums")
            nc.vector.tensor_copy(tsums, acc2)
            nc.sync.dma_start(out=tsum, in_=tsums)

    nc.compile()
    return nc


def _zcols():
    zc = np.zeros((P, P), dtype=np.float32)
    zc[:, 64] = 1.0
    return zc


def _sign_class_mask():
    """Boolean [C]: classes whose selection stream went through the
    Sign path (need the +T_half shift restored)."""
    msk = np.zeros(C, dtype=bool)
    for j in range(NXCH):
        msk[j * XCH : j * XCH + SIGN_WINDOWS[j] * WCH] = True
    return msk


def _postprocess(sums_2d, tsum_row):
    sums = np.asarray(sums_2d, dtype=np.float64).reshape(-1)
    t_half = float(np.asarray(tsum_row, dtype=np.float64).sum())
    sums[_sign_class_mask()] += t_half
    return sums


def kernel(**inputs):
    from concourse import bass_utils

    x = np.asarray(inputs["inputs"], dtype=np.float32)
    assert x.shape == (B, S, C), x.shape

    if "nc" not in _CACHE:
        _CACHE["nc"] = _build_graph()
    nc = _CACHE["nc"]

    zc = _zcols()
    in_maps = [
        {"x": np.ascontiguousarray(x[b]), "zcols": zc} for b in range(B)
    ]
    res = bass_utils.run_bass_kernel_spmd(nc, in_maps, core_ids=list(range(B)))

    out = np.empty((B,), dtype=np.int32)
    for b in range(B):
        sums = _postprocess(res.results[b]["out"], res.results[b]["tsum"])
        out[b] = np.argmax(sums)
    return out


# revision 18
# speedup vs baseline: 1.0246x; 1.0246x over previous
"""ArgMaxTop Trainium2 kernel.

Math: out[b] = argmax_c sum_s x[b,s,c] * [x[b,s,c] >= t(b,s)] where t is the
8th-largest value of row (b,s). This equals the reference's
scatter-top8/mean/argmax pipeline for inputs without exact float ties (the
mean-over-s divides every class by the same S, so argmax is unchanged; absent
classes sum to 0 < any winner).

Sharding: batch b -> core b (8 batches, 8 cores), no collectives.

Per-core dataflow, per 128-row s-tile (16 tiles):
  - DMA x chunks [128, 8000] f32 to SBUF (pool of 5 bufs for overlap)
  - DVE `max` (top-8 per partition) per chunk -> concat [128,32] -> max -> t
  - value stream (all classes, ACT): r = Relu(x - t), f32r
  - selection stream, split across engines to balance them (DVE also owns
    the max8 pass; GpSimd is a DSP, useless for elementwise):
      * DVE windows: m2 = (x >= t) * t, f32r (fused tensor_scalar, 2x mode)
      * ACT windows: s = Sign(x - t') in f32r ({-1,+1}, exact); t' =
        fl(t*(1-2^-23)) sits strictly between the 9th and 8th largest for
        every row of this input (verified: no x in [t', t)), so s = 2m-1
        exactly; contracted against an f32r t/2 stationary this contributes
        t*m - t/2 per row. The -t/2 shift is class-independent; it is
        measured per tile (tsum output) and added back on the host.
  - PE: per 500-wide class window j, two matmuls accumulate into PSUM
    [64,500] via shifted-window one-hot stationaries (ones in column 64):
      psum += ones_j^T @ r  +  Wsel_j^T @ sel
    where Wsel = ones for m2 windows and the per-tile t/2 column (W2,
    double-buffered by tile parity) for sign windows.
  - drain psum -> SBUF -> DRAM out [64, 500] + tsum [1,16] (per-tile
    sum_s t_s/2, fp32); host adds T_half to the sign-path classes and
    argmaxes the 32000 sums.
"""

import sys

if "/opt/trn_rl_repo" not in sys.path:
    sys.path.insert(0, "/opt/trn_rl_repo")

import numpy as np

B, S, C = 8, 2048, 32000
TOP_K = 8
P = 128          # partitions per s-tile
XCH = 8000       # x chunk width (DVE max input free size <= 16384)
WCH = 2000       # relu/sign/mask window width
RCH = 1000       # relu (ACT) window width
CCH = 500        # matmul moving window / psum columns
NROWS = C // CCH  # 64 psum rows
NTILES = S // P
NXCH = C // XCH
NW = XCH // WCH  # 4 select windows per x chunk
# per chunk j: sign path covers the first SIGN_WINDOWS[j] of its 4 windows
SIGN_WINDOWS = [0, 0, 0, 0]

_CACHE = {}


def _build_graph(s_len=S, x_bufs=5):
    from concourse import bacc, tile, mybir

    f32 = mybir.dt.float32
    f32r = mybir.dt.float32r
    Alu = mybir.AluOpType
    Act = mybir.ActivationFunctionType

    nc = bacc.Bacc("TRN2", target_bir_lowering=False, debug=False)
    x = nc.dram_tensor("x", [s_len, C], f32, kind="ExternalInput").ap()
    zc = nc.dram_tensor("zcols", [P, P], f32, kind="ExternalInput").ap()
    out = nc.dram_tensor("out", [NROWS, CCH], f32, kind="ExternalOutput").ap()
    ntiles = s_len // P
    tsum = nc.dram_tensor("tsum", [1, ntiles], f32, kind="ExternalOutput").ap()

    n_mm = ntiles * (C // CCH) * 2
    mm_i = 0
    SCALE_P = float(np.float32(1.0) - np.float32(2.0**-23))

    with tile.TileContext(nc) as tc:
        with (
            tc.tile_pool(name="consts", bufs=1) as consts,
            tc.tile_pool(name="xp", bufs=x_bufs) as xp,
            tc.tile_pool(name="tp", bufs=3) as tp,
            tc.tile_pool(name="rp", bufs=3) as rp,
            tc.tile_pool(name="selp", bufs=3) as selp,
            tc.tile_pool(name="sump", bufs=1) as sump,
            tc.tile_pool(name="ps", bufs=1, space="PSUM") as ps,
            tc.tile_pool(name="ps2", bufs=1, space="PSUM") as ps2,
        ):
            zt = consts.tile([P, P], f32, name="zt")
            nc.sync.dma_start(out=zt, in_=zc)
            ztm = consts.tile([P, P], f32r, name="ztm")
            nc.vector.tensor_copy(ztm, zt)
            w2 = []
            if any(SIGN_WINDOWS):
                w2 = [
                    consts.tile([P, P], f32r, name=f"w2_{k}")
                    for k in range(2)
                ]
                for w in w2:
                    # zeros except col 64, which every tile overwrites with
                    # t/2 before use (f32r memset fails the ISA check)
                    nc.vector.tensor_copy(w, zt)

            acc = ps.tile([NROWS, CCH], f32, name="acc")
            acc2 = ps2.tile([1, ntiles], f32, name="acc2")

            for it in range(ntiles):
                xch = []
                for j in range(NXCH):
                    xt = xp.tile([P, XCH], f32, name="xch", tag="xch")
                    nc.sync.dma_start(
                        out=xt,
                        in_=x[it * P : (it + 1) * P, j * XCH : (j + 1) * XCH],
                    )
                    xch.append(xt)

                top = tp.tile([P, 8 * NXCH], f32, name="top", tag="top")
                for j in range(NXCH):
                    nc.vector.max(out=top[:, 8 * j : 8 * (j + 1)], in_=xch[j])
                top8 = tp.tile([P, 8], f32, name="top8", tag="top8")
                nc.vector.max(out=top8, in_=top)
                t_ap = top8[:, 7:8]

                # per-tile scalars: relu bias, sign bias, t/2 column
                tneg = tp.tile([P, 1], f32, name="tneg", tag="tneg")
                nc.vector.tensor_scalar(tneg, t_ap, -1.0, None, Alu.mult)
                if any(SIGN_WINDOWS):
                    tpneg = tp.tile([P, 1], f32, name="tpneg", tag="tpneg")
                    nc.scalar.activation(
                        out=tpneg, in_=t_ap, func=Act.Copy, scale=-SCALE_P
                    )
                    t2col = w2[it % 2][:, 64:65]
                    nc.scalar.activation(
                        out=t2col, in_=t_ap, func=Act.Copy, scale=0.5
                    )
                    # per-tile T_half = sum_s t_s/2 (own psum column, host-
                    # summed; fp32 1-col matmul — f32r rejects 1-wide moving)
                    t2f = tp.tile([P, 1], f32, name="t2f", tag="t2f")
                    nc.scalar.activation(out=t2f, in_=t2col, func=Act.Copy)
                    nc.tensor.matmul(
                        acc2[:, it : it + 1],
                        zt[:, 64:65],
                        t2f,
                        start=True,
                        stop=True,
                    )
                else:
                    nc.tensor.matmul(
                        acc2[:, it : it + 1],
                        zt[:, 64:65],
                        tneg,
                        start=True,
                        stop=True,
                    )

                for j in range(NXCH):
                    nsign = SIGN_WINDOWS[j]
                    for wi in range(NW):
                        xs = xch[j][:, wi * WCH : (wi + 1) * WCH]
                        if wi < nsign:
                            sel = selp.tile(
                                [P, WCH], f32r, name="sel", tag="sel"
                            )
                            nc.scalar.activation(
                                out=sel,
                                in_=xs,
                                func=Act.Sign,
                                bias=tpneg,
                                scale=1.0,
                            )
                            wsel = w2[it % 2]
                        else:
                            sel = selp.tile(
                                [P, WCH], f32r, name="sel", tag="sel"
                            )
                            nc.vector.tensor_scalar(
                                sel, xs, t_ap, t_ap, Alu.is_ge, Alu.mult
                            )
                            wsel = ztm
                        rtiles = []
                        for rs in range(WCH // RCH):
                            off = wi * WCH + rs * RCH
                            r = rp.tile([P, RCH], f32r, name="r", tag="r")
                            nc.scalar.activation(
                                out=r,
                                in_=xch[j][:, off : off + RCH],
                                func=Act.Relu,
                                bias=tneg,
                                scale=1.0,
                            )
                            rtiles.append(r)
                        for w in range(WCH // CCH):
                            cg = (j * XCH + wi * WCH + w * CCH) // CCH
                            off = w * CCH
                            rsrc = rtiles[off // RCH][
                                :, off % RCH : off % RCH + CCH
                            ]
                            nc.tensor.matmul(
                                acc,
                                ztm[:, 64 - cg : 128 - cg],
                                rsrc,
                                start=(mm_i == 0),
                                stop=(mm_i == n_mm - 1),
                            )
                            mm_i += 1
                            nc.tensor.matmul(
                                acc,
                                wsel[:, 64 - cg : 128 - cg],
                                sel[:, off : off + CCH],
                                start=False,
                                stop=(mm_i == n_mm - 1),
                            )
                            mm_i += 1

            sums = sump.tile([NROWS, CCH], f32, name="sums")
            nc.vector.tensor_copy(sums, acc)
            nc.sync.dma_start(out=out, in_=sums)
            tsums = sump.tile([1, ntiles], f32, name="tsums")
            nc.vector.tensor_copy(tsums, acc2)
            nc.sync.dma_start(out=tsum, in_=tsums)

    nc.compile()
    return nc


def _zcols():
    zc = np.zeros((P, P), dtype=np.float32)
    zc[:, 64] = 1.0
    return zc


def _sign_class_mask():
    """Boolean [C]: classes whose selection stream went through the
    Sign path (need the +T_half shift restored)."""
    msk = np.zeros(C, dtype=bool)
    for j in range(NXCH):
        msk[j * XCH : j * XCH + SIGN_WINDOWS[j] * WCH] = True
    return msk


def _postprocess(sums_2d, tsum_row):
    sums = np.asarray(sums_2d, dtype=np.float64).reshape(-1)
    t_half = float(np.asarray(tsum_row, dtype=np.float64).sum())
    sums[_sign_class_mask()] += t_half
    return sums


def kernel(**inputs):
    from concourse import bass_utils

    x = np.asarray(inputs["inputs"], dtype=np.float32)
    assert x.shape == (B, S, C), x.shape

    if "nc" not in _CACHE:
        _CACHE["nc"] = _build_graph()
    nc = _CACHE["nc"]

    zc = _zcols()
    in_maps = [
        {"x": np.ascontiguousarray(x[b]), "zcols": zc} for b in range(B)
    ]
    res = bass_utils.run_bass_kernel_spmd(nc, in_maps, core_ids=list(range(B)))

    out = np.empty((B,), dtype=np.int32)
    for b in range(B):
        sums = _postprocess(res.results[b]["out"], res.results[b]["tsum"])
        out[b] = np.argmax(sums)
    return out


# revision 19
# speedup vs baseline: 1.0440x; 1.0189x over previous
"""ArgMaxTop Trainium2 kernel.

Math: out[b] = argmax_c sum_s x[b,s,c] * [x[b,s,c] >= t(b,s)] where t is the
8th-largest value of row (b,s). This equals the reference's
scatter-top8/mean/argmax pipeline for inputs without exact float ties (the
mean-over-s divides every class by the same S, so argmax is unchanged; absent
classes sum to 0 < any winner).

Sharding: batch b -> core b (8 batches, 8 cores), no collectives.

Per-core dataflow, per 128-row s-tile (16 tiles):
  - DMA x chunks [128, 8000] f32 to SBUF (pool of 5 bufs for overlap)
  - DVE `max` (top-8 per partition) per chunk -> concat [128,32] -> max -> t
  - value stream (all classes, ACT): r = Relu(x - t), f32r
  - selection stream, split across engines to balance them (DVE also owns
    the max8 pass; GpSimd is a DSP, useless for elementwise):
      * DVE windows: m2 = (x >= t) * t, f32r (fused tensor_scalar, 2x mode)
      * ACT windows: s = Sign(x - t') in f32r ({-1,+1}, exact); t' =
        fl(t*(1-2^-23)) sits strictly between the 9th and 8th largest for
        every row of this input (verified: no x in [t', t)), so s = 2m-1
        exactly; contracted against an f32r t/2 stationary this contributes
        t*m - t/2 per row. The -t/2 shift is class-independent; it is
        measured per tile (tsum output) and added back on the host.
  - PE: per 500-wide class window j, two matmuls accumulate into PSUM
    [64,500] via shifted-window one-hot stationaries (ones in column 64):
      psum += ones_j^T @ r  +  Wsel_j^T @ sel
    where Wsel = ones for m2 windows and the per-tile t/2 column (W2,
    double-buffered by tile parity) for sign windows.
  - drain psum -> SBUF -> DRAM out [64, 500] + tsum [1,16] (per-tile
    sum_s t_s/2, fp32); host adds T_half to the sign-path classes and
    argmaxes the 32000 sums.
"""

import sys

if "/opt/trn_rl_repo" not in sys.path:
    sys.path.insert(0, "/opt/trn_rl_repo")

import numpy as np

B, S, C = 8, 2048, 32000
TOP_K = 8
P = 128          # partitions per s-tile
XCH = 8000       # x chunk width (DVE max input free size <= 16384)
WCH = 4000       # mask window width
RCH = 1000       # relu (ACT) window width
CCH = 500        # matmul moving window / psum columns
NROWS = C // CCH  # 64 psum rows
NTILES = S // P
NXCH = C // XCH
NW = XCH // WCH  # 4 select windows per x chunk
# per chunk j: sign path covers the first SIGN_WINDOWS[j] of its 4 windows
SIGN_WINDOWS = [0, 0, 0, 0]

_CACHE = {}


def _build_graph(s_len=S, x_bufs=5):
    from concourse import bacc, tile, mybir

    f32 = mybir.dt.float32
    f32r = mybir.dt.float32r
    Alu = mybir.AluOpType
    Act = mybir.ActivationFunctionType

    nc = bacc.Bacc("TRN2", target_bir_lowering=False, debug=False)
    x = nc.dram_tensor("x", [s_len, C], f32, kind="ExternalInput").ap()
    zc = nc.dram_tensor("zcols", [P, P], f32, kind="ExternalInput").ap()
    out = nc.dram_tensor("out", [NROWS, CCH], f32, kind="ExternalOutput").ap()
    ntiles = s_len // P
    tsum = nc.dram_tensor("tsum", [1, ntiles], f32, kind="ExternalOutput").ap()

    n_mm = ntiles * (C // CCH) * 2
    mm_i = 0
    SCALE_P = float(np.float32(1.0) - np.float32(2.0**-23))

    with tile.TileContext(nc) as tc:
        with (
            tc.tile_pool(name="consts", bufs=1) as consts,
            tc.tile_pool(name="xp", bufs=x_bufs) as xp,
            tc.tile_pool(name="tp", bufs=3) as tp,
            tc.tile_pool(name="rp", bufs=3) as rp,
            tc.tile_pool(name="selp", bufs=2) as selp,
            tc.tile_pool(name="sump", bufs=1) as sump,
            tc.tile_pool(name="ps", bufs=1, space="PSUM") as ps,
            tc.tile_pool(name="ps2", bufs=1, space="PSUM") as ps2,
        ):
            zt = consts.tile([P, P], f32, name="zt")
            nc.sync.dma_start(out=zt, in_=zc)
            ztm = consts.tile([P, P], f32r, name="ztm")
            nc.vector.tensor_copy(ztm, zt)
            w2 = []
            if any(SIGN_WINDOWS):
                w2 = [
                    consts.tile([P, P], f32r, name=f"w2_{k}")
                    for k in range(2)
                ]
                for w in w2:
                    # zeros except col 64, which every tile overwrites with
                    # t/2 before use (f32r memset fails the ISA check)
                    nc.vector.tensor_copy(w, zt)

            acc = ps.tile([NROWS, CCH], f32, name="acc")
            acc2 = ps2.tile([1, ntiles], f32, name="acc2")

            for it in range(ntiles):
                xch = []
                for j in range(NXCH):
                    xt = xp.tile([P, XCH], f32, name="xch", tag="xch")
                    nc.sync.dma_start(
                        out=xt,
                        in_=x[it * P : (it + 1) * P, j * XCH : (j + 1) * XCH],
                    )
                    xch.append(xt)

                top = tp.tile([P, 8 * NXCH], f32, name="top", tag="top")
                for j in range(NXCH):
                    nc.vector.max(out=top[:, 8 * j : 8 * (j + 1)], in_=xch[j])
                top8 = tp.tile([P, 8], f32, name="top8", tag="top8")
                nc.vector.max(out=top8, in_=top)
                t_ap = top8[:, 7:8]

                # per-tile scalars: relu bias, sign bias, t/2 column
                tneg = tp.tile([P, 1], f32, name="tneg", tag="tneg")
                nc.vector.tensor_scalar(tneg, t_ap, -1.0, None, Alu.mult)
                if any(SIGN_WINDOWS):
                    tpneg = tp.tile([P, 1], f32, name="tpneg", tag="tpneg")
                    nc.scalar.activation(
                        out=tpneg, in_=t_ap, func=Act.Copy, scale=-SCALE_P
                    )
                    t2col = w2[it % 2][:, 64:65]
                    nc.scalar.activation(
                        out=t2col, in_=t_ap, func=Act.Copy, scale=0.5
                    )
                    # per-tile T_half = sum_s t_s/2 (own psum column, host-
                    # summed; fp32 1-col matmul — f32r rejects 1-wide moving)
                    t2f = tp.tile([P, 1], f32, name="t2f", tag="t2f")
                    nc.scalar.activation(out=t2f, in_=t2col, func=Act.Copy)
                    nc.tensor.matmul(
                        acc2[:, it : it + 1],
                        zt[:, 64:65],
                        t2f,
                        start=True,
                        stop=True,
                    )
                else:
                    nc.tensor.matmul(
                        acc2[:, it : it + 1],
                        zt[:, 64:65],
                        tneg,
                        start=True,
                        stop=True,
                    )

                for j in range(NXCH):
                    nsign = SIGN_WINDOWS[j]
                    for wi in range(NW):
                        xs = xch[j][:, wi * WCH : (wi + 1) * WCH]
                        if wi < nsign:
                            sel = selp.tile(
                                [P, WCH], f32r, name="sel", tag="sel"
                            )
                            nc.scalar.activation(
                                out=sel,
                                in_=xs,
                                func=Act.Sign,
                                bias=tpneg,
                                scale=1.0,
                            )
                            wsel = w2[it % 2]
                        else:
                            sel = selp.tile(
                                [P, WCH], f32r, name="sel", tag="sel"
                            )
                            nc.vector.tensor_scalar(
                                sel, xs, t_ap, t_ap, Alu.is_ge, Alu.mult
                            )
                            wsel = ztm
                        rtiles = []
                        for rs in range(WCH // RCH):
                            off = wi * WCH + rs * RCH
                            r = rp.tile([P, RCH], f32r, name="r", tag="r")
                            nc.scalar.activation(
                                out=r,
                                in_=xch[j][:, off : off + RCH],
                                func=Act.Relu,
                                bias=tneg,
                                scale=1.0,
                            )
                            rtiles.append(r)
                        for w in range(WCH // CCH):
                            cg = (j * XCH + wi * WCH + w * CCH) // CCH
                            off = w * CCH
                            rsrc = rtiles[off // RCH][
                                :, off % RCH : off % RCH + CCH
                            ]
                            nc.tensor.matmul(
                                acc,
                                ztm[:, 64 - cg : 128 - cg],
                                rsrc,
                                start=(mm_i == 0),
                                stop=(mm_i == n_mm - 1),
                            )
                            mm_i += 1
                            nc.tensor.matmul(
                                acc,
                                wsel[:, 64 - cg : 128 - cg],
                                sel[:, off : off + CCH],
                                start=False,
                                stop=(mm_i == n_mm - 1),
                            )
                            mm_i += 1

            sums = sump.tile([NROWS, CCH], f32, name="sums")
            nc.vector.tensor_copy(sums, acc)
            nc.sync.dma_start(out=out, in_=sums)
            tsums = sump.tile([1, ntiles], f32, name="tsums")
            nc.vector.tensor_copy(tsums, acc2)
            nc.sync.dma_start(out=tsum, in_=tsums)

    nc.compile()
    return nc


def _zcols():
    zc = np.zeros((P, P), dtype=np.float32)
    zc[:, 64] = 1.0
    return zc


def _sign_class_mask():
    """Boolean [C]: classes whose selection stream went through the
    Sign path (need the +T_half shift restored)."""
    msk = np.zeros(C, dtype=bool)
    for j in range(NXCH):
        msk[j * XCH : j * XCH + SIGN_WINDOWS[j] * WCH] = True
    return msk


def _postprocess(sums_2d, tsum_row):
    sums = np.asarray(sums_2d, dtype=np.float64).reshape(-1)
    t_half = float(np.asarray(tsum_row, dtype=np.float64).sum())
    sums[_sign_class_mask()] += t_half
    return sums


def kernel(**inputs):
    from concourse import bass_utils

    x = np.asarray(inputs["inputs"], dtype=np.float32)
    assert x.shape == (B, S, C), x.shape

    if "nc" not in _CACHE:
        _CACHE["nc"] = _build_graph()
    nc = _CACHE["nc"]

    zc = _zcols()
    in_maps = [
        {"x": np.ascontiguousarray(x[b]), "zcols": zc} for b in range(B)
    ]
    res = bass_utils.run_bass_kernel_spmd(nc, in_maps, core_ids=list(range(B)))

    out = np.empty((B,), dtype=np.int32)
    for b in range(B):
        sums = _postprocess(res.results[b]["out"], res.results[b]["tsum"])
        out[b] = np.argmax(sums)
    return out
